# revision 13
# baseline (speedup 1.0000x reference)
"""Multi-head causal self-attention on 8 Trainium2 NeuronCores.

Problem: B=2, S=2048, D=1024, H=16 heads (dk=64), causal softmax attention,
fp32 in/out.  y = softmax(mask(Q K^T / sqrt(dk))) V  projected by Wo.

Sharding (no device-to-device communication needed):
  core c -> batch b = c // 4, head-group hg = c % 4 (4 heads = 256 dims each).
  Each core computes its 4 heads' attention output and a *partial* final
  projection (its 256 rows of the Wo contraction).  The host sums the 4
  partials per batch and stacks the 2 batches.

On-device strategy (per core):
  All matmul operands live in SBUF with the contraction dim on partitions;
  the host passes x^T and W^T slices so no on-device transposes are needed.
  Scores are computed transposed (S^T = K Q^T, keys on partitions) so that
  P^T = exp(S^T) is directly the stationary-side layout the PV matmul needs;
  softmax normalization moves to the *output* side (divide the 64-dim head
  output by the row sum), with row sums produced for free by an extra
  ones-column appended to V.  All matmuls run as float32r (full PE rate,
  ~1e-3 operand rounding).  Causal masking is tile-level: strictly-upper
  tiles are skipped entirely; diagonal tiles are zeroed post-exp by a gpsimd
  affine_select.  Head pairs live on partitions 0-63 / 64-127 so the K=64
  score matmuls can pack into the PE array's row-tiling, and both heads'
  score tiles share one 2-bank PSUM tile so a single wide exp serves both.

  The program is a software pipeline over 512-query chunks: the x^T DMA is
  issued in per-chunk slices, chunk 0's projections run as soon as its slice
  lands, and every later chunk's projections (K/Q/V) plus the previous
  chunk's output projection are drip-fed into the attention instruction
  stream one item per k-iteration, so neither the PE nor the ACT engine
  ever stalls behind a monolithic phase.
"""

import sys

sys.path.insert(0, "/opt/trn_rl_repo")

import numpy as np

import concourse.bacc as bacc
import concourse.mybir as mybir
import concourse.tile as tile
from concourse.bass_utils import run_bass_kernel_spmd

F32 = mybir.dt.float32
F32R = mybir.dt.float32r
EXPF = mybir.ActivationFunctionType.Exp

B, S, D, H = 2, 2048, 1024, 16
DK = D // H          # 64
E = 256              # head dims per core (4 heads)
HL = 4               # local heads per core
QC = 512             # query chunk (free dim of S^T tiles)
NKT = S // 128       # 16 key tiles
NQC = S // QC        # 4
NDT = D // 128       # 8 contraction tiles for projections
NDH = NDT // 2       # half-contraction split for DMA overlap
SCALE = float(1.0 / np.sqrt(np.float32(DK)))


def _build_nc(repeats=1, phases=("att", "fin")):
    nc = bacc.Bacc("TRN2", target_bir_lowering=False, debug=False)

    xT = nc.dram_tensor("xT", [D, S], F32R, kind="ExternalInput")
    wqT = nc.dram_tensor("wqT", [D, E], F32R, kind="ExternalInput")
    wkT = nc.dram_tensor("wkT", [D, E], F32R, kind="ExternalInput")
    wvT = nc.dram_tensor("wvT", [D, E], F32R, kind="ExternalInput")
    woT = nc.dram_tensor("woT", [E, D], F32R, kind="ExternalInput")
    out = nc.dram_tensor("out", [S, D], F32, kind="ExternalOutput")

    with tile.TileContext(nc) as tc:
        with (
            tc.tile_pool(name="const", bufs=1) as const,
            tc.tile_pool(name="work", bufs=3) as work,
            tc.tile_pool(name="outp", bufs=4) as outp,
            tc.tile_pool(name="norm", bufs=3) as norm,
            tc.tile_pool(name="psmm", bufs=2, space="PSUM") as psmm,
            tc.tile_pool(name="psst", bufs=2, space="PSUM") as psst,
            tc.tile_pool(name="pspv", bufs=2, space="PSUM") as pspv,
        ):
            for _rep in range(repeats):
                # ---- resident tensors ---------------------------------
                xT_sb = const.tile([128, NDT, S], F32R, tag="xT")
                wq_sb = const.tile([128, NDT, E], F32R, tag="wq")
                wk_sb = const.tile([128, NDT, E], F32R, tag="wk")
                wv_sb = const.tile([128, NDT, E], F32R, tag="wv")
                wo_sb = const.tile([128, 2, D], F32R, tag="wo")
                qT_sb = const.tile([128, 2, S], F32R, tag="qT")
                kT_sb = const.tile([128, 2, S], F32R, tag="kT")
                v_sb = const.tile([128, NKT, HL * (DK + 1)], F32R, tag="v")
                oT_sb = const.tile([128, 2, S], F32R, tag="oT")
                zeros = const.tile([128, HL * NKT], F32, tag="zeros")
                nc.vector.memset(zeros[:], 0.0)

                # ---- DMA in: x^T in per-chunk slices ------------------
                xT_r = xT.rearrange("(k p) s -> p k s", p=128)
                for w_sb, w_dr in ((wk_sb, wkT), (wq_sb, wqT)):
                    nc.sync.dma_start(
                        w_sb[:], w_dr.rearrange("(k p) e -> p k e", p=128))
                for kt in range(NDT):
                    nc.sync.dma_start(xT_sb[:, kt, 0:QC], xT_r[:, kt, 0:QC])
                nc.sync.dma_start(
                    wv_sb[:], wvT.rearrange("(k p) e -> p k e", p=128))
                for sc in range(1, NQC):
                    for kt in range(NDT):
                        nc.sync.dma_start(
                            xT_sb[:, kt, sc * QC:(sc + 1) * QC],
                            xT_r[:, kt, sc * QC:(sc + 1) * QC])
                nc.sync.dma_start(
                    wo_sb[:], woT.rearrange("(g p) e -> p g e", p=128))

                v_h = v_sb.rearrange("p t (h x) -> p t h x", h=HL)
                # ones columns for all 16 V tiles in one shot: exp(0) = 1
                nc.scalar.activation(
                    v_h[:, :, :, DK:DK + 1],
                    zeros[:].rearrange("p (t h o) -> p t h o", h=HL, o=1),
                    EXPF, scale=0.0,
                )

                # ---- emit helpers -------------------------------------
                def emit_proj(nm, w_sb, t_sb, g, sc, hf):
                    # K^T/Q^T chunk: [e_local, s-chunk] = W^T.T @ x^T,
                    # contraction split in d-tile halves for DMA overlap
                    k0, k1 = (0, NDH) if hf == 0 else (NDH, NDT)
                    tgt = t_sb[:, g, sc * QC:(sc + 1) * QC]
                    ps = psmm.tile([128, QC], F32, tag="mm",
                                   name=f"pj{hf}_{nm}_{g}_{sc}_{_rep}")
                    for kt in range(k0, k1):
                        nc.tensor.matmul(
                            ps[:],
                            w_sb[:, kt, g * 128:(g + 1) * 128],
                            xT_sb[:, kt, sc * QC:(sc + 1) * QC],
                            start=(kt == k0), stop=(kt == k1 - 1),
                        )
                    if hf == 0:
                        nc.vector.tensor_copy(tgt, ps[:])
                    else:
                        nc.vector.tensor_add(tgt, ps[:], tgt)

                def emit_v_tile(st):
                    # V natural [s, e_local] (ones columns pre-written)
                    ps = psmm.tile([128, QC], F32, tag="mm",
                                   name=f"v_{st}_{_rep}")
                    for kt in range(NDT):
                        nc.tensor.matmul(
                            ps[:, 0:E],
                            xT_sb[:, kt, st * 128:(st + 1) * 128],
                            wv_sb[:, kt, :],
                            start=(kt == 0), stop=(kt == NDT - 1),
                        )
                    nc.vector.tensor_copy(
                        v_h[:, st, :, 0:DK],
                        ps[:, 0:E].rearrange("p (h d) -> p h d", h=HL),
                    )

                def emit_final(st, ec):
                    fp = psmm.tile([128, QC], F32, tag="mm",
                                   name=f"f_{st}_{ec}_{_rep}")
                    for g in range(2):
                        nc.tensor.matmul(
                            fp[:],
                            oT_sb[:, g, st * 128:(st + 1) * 128],
                            wo_sb[:, g, ec * QC:(ec + 1) * QC],
                            start=(g == 0), stop=(g == 1),
                        )
                    fsb = outp.tile([128, QC], F32, tag="fsb")
                    nc.vector.tensor_copy(fsb[:], fp[:])
                    nc.sync.dma_start(
                        out[st * 128:(st + 1) * 128, ec * QC:(ec + 1) * QC],
                        fsb[:],
                    )

                v_next = [0]

                def ensure_v(st_needed):
                    while v_next[0] <= st_needed:
                        emit_v_tile(v_next[0])
                        v_next[0] += 1

                def emit_slice(sc, with_v=True):
                    # all projection inputs attention chunk sc depends on
                    for hf in range(2):
                        for nm, w_sb, t_sb in (("k", wk_sb, kT_sb),
                                               ("q", wq_sb, qT_sb)):
                            for g in range(2):
                                emit_proj(nm, w_sb, t_sb, g, sc, hf)
                    if with_v:
                        ensure_v(4 * sc + 3)

                # deferred PE filler: (due_chunk, emit_fn); drained one item
                # per k-iteration, force-flushed at its due chunk's start
                deferred = []

                def drain_one():
                    if deferred:
                        deferred.pop(0)[1]()

                def flush_due(c):
                    while deferred and deferred[0][0] <= c:
                        deferred.pop(0)[1]()

                # ---- prologue: chunk 0 inputs (V tiles 1-3 arrive
                # lazily inside the first k-loop) ------------------------
                emit_slice(0, with_v=False)
                ensure_v(0)

                n_chunks = NQC if "att" in phases else 0
                for c in range(n_chunks):
                    if c + 1 < NQC:
                        for hf in range(2):
                            for nm, w_sb, t_sb in (("k", wk_sb, kT_sb),
                                                   ("q", wq_sb, qT_sb)):
                                for g in range(2):
                                    deferred.append((c + 1, lambda nm=nm,
                                                     w=w_sb, t=t_sb, g=g,
                                                     sc=c + 1, hf=hf:
                                                     emit_proj(nm, w, t, g,
                                                               sc, hf)))
                        for st in range(4 * (c + 1), 4 * (c + 1) + 4):
                            deferred.append(
                                (c + 1, lambda st=st: ensure_v(st)))
                    if c > 0 and "fin" in phases:
                        for st in range(4 * (c - 1), 4 * (c - 1) + 4):
                            for ec in range(2):
                                deferred.append(
                                    (99, lambda st=st, ec=ec:
                                     emit_final(st, ec)))
                    flush_due(c)
                    for g in range(2):
                        pv_ps = {}
                        n_kt = 4 * c + 4
                        for kt in range(n_kt):
                            diag = kt >= 4 * c
                            j = kt - 4 * c
                            w = min(j, 2) * 128 if diag else 0
                            if diag:
                                ensure_v(kt)
                            # both heads' S^T in one 2-bank psum tile; the
                            # K=64 matmuls pack into PE row-tiling
                            wide = psst.tile([128, 2, QC], F32, tag="st2",
                                             name=f"st_{c}_{g}_{kt}_{_rep}")
                            for li in range(2):
                                r0 = li * 64
                                nc.tensor.matmul(
                                    wide[:, li, w:QC],
                                    kT_sb[r0:r0 + 64, g,
                                          kt * 128:(kt + 1) * 128],
                                    qT_sb[r0:r0 + 64, g,
                                          c * QC + w:(c + 1) * QC],
                                    start=True, stop=True,
                                )
                            ptw = work.tile([128, 2, QC], F32R, tag="pt")
                            if not diag:
                                nc.scalar.activation(
                                    ptw[:], wide[:], EXPF, scale=SCALE)
                            else:
                                for li in range(2):
                                    nc.scalar.activation(
                                        ptw[:, li, w:QC], wide[:, li, w:QC],
                                        EXPF, scale=SCALE)
                                for li in range(2):
                                    if j < 3:
                                        nc.gpsimd.affine_select(
                                            out=ptw[:, li,
                                                    j * 128:(j + 1) * 128],
                                            in_=ptw[:, li,
                                                    j * 128:(j + 1) * 128],
                                            compare_op=mybir.AluOpType.is_ge,
                                            fill=0.0, base=0,
                                            pattern=[[1, 128]],
                                            channel_multiplier=-1,
                                        )
                                    else:
                                        nc.gpsimd.affine_select(
                                            out=ptw[:, li, 256:512],
                                            in_=ptw[:, li, 256:512],
                                            compare_op=mybir.AluOpType.is_ge,
                                            fill=0.0, base=-128,
                                            pattern=[[1, 256]],
                                            channel_multiplier=-1,
                                        )
                            for li in range(2):
                                h = 2 * g + li
                                if kt == 0:
                                    pv_ps[li] = pspv.tile(
                                        [128, QC], F32, tag="pv",
                                        name=f"pv_{c}_{g}_{li}_{_rep}")
                                nc.tensor.matmul(
                                    pv_ps[li][0:DK + 1, w:QC],
                                    v_sb[:, kt,
                                         h * (DK + 1):(h + 1) * (DK + 1)],
                                    ptw[:, li, w:QC],
                                    start=(kt == 0), stop=(kt == n_kt - 1),
                                )
                            drain_one()
                            if c < 2:
                                drain_one()
                        # normalize: oT[head rows, c] = pv[0:64]/pv[64]
                        for li in range(2):
                            r0 = li * 64
                            rc = norm.tile([1, QC], F32, tag="rc")
                            nc.vector.reciprocal(
                                rc[0:1, :], pv_ps[li][DK:DK + 1, :])
                            rbc = norm.tile([64, QC], F32, tag="rbc")
                            nc.gpsimd.partition_broadcast(rbc[:], rc[0:1, :])
                            nc.vector.tensor_mul(
                                oT_sb[r0:r0 + 64, g, c * QC:(c + 1) * QC],
                                pv_ps[li][0:DK, :],
                                rbc[:],
                            )
                while deferred:
                    deferred.pop(0)[1]()
                if "att" not in phases:
                    for sc in range(1, NQC):
                        emit_slice(sc)
                    ensure_v(NKT - 1)
                if "fin" in phases and "att" in phases:
                    for st in range(12, 16):
                        for ec in range(2):
                            emit_final(st, ec)
                if "fin" not in phases:
                    cons = outp.tile([128, QC], F32, tag="fsb",
                                     name=f"cons_{_rep}")
                    nc.vector.tensor_copy(cons[:, 0:128], qT_sb[:, 0, 0:128])
                    nc.vector.tensor_copy(cons[:, 128:256],
                                          kT_sb[:, 1, 0:128])
                    nc.vector.tensor_copy(cons[:, 256:260], v_sb[:, 3, 0:4])
                    if "att" in phases:
                        nc.vector.tensor_copy(cons[:, 260:380],
                                              oT_sb[:, 0, 0:120])
                    nc.sync.dma_start(out[0:128, 0:QC], cons[:])

    nc.compile()
    return nc


_NC = None


def _get_nc():
    global _NC
    if _NC is None:
        _NC = _build_nc()
    return _NC


def _in_maps(x, Wq, Wk, Wv, Wo):
    x, Wq, Wk, Wv, Wo = (np.asarray(a, dtype=np.float32)
                         for a in (x, Wq, Wk, Wv, Wo))
    maps = []
    for c in range(8):
        b, hg = divmod(c, 4)
        sl = slice(hg * E, (hg + 1) * E)
        maps.append({
            "xT": np.ascontiguousarray(x[b].T),
            "wqT": np.ascontiguousarray(Wq[sl].T),
            "wkT": np.ascontiguousarray(Wk[sl].T),
            "wvT": np.ascontiguousarray(Wv[sl].T),
            "woT": np.ascontiguousarray(Wo[:, sl].T),
        })
    return maps


def kernel(x, Wq, Wk, Wv, Wo, _trace=False, _trace_kwargs=None):
    nc = _get_nc()
    maps = _in_maps(x, Wq, Wk, Wv, Wo)
    res = run_bass_kernel_spmd(
        nc, maps, core_ids=list(range(8)),
        trace=_trace, **(_trace_kwargs or {}),
    )
    outs = [res.results[c]["out"] for c in range(8)]
    full = np.stack([
        outs[0] + outs[1] + outs[2] + outs[3],
        outs[4] + outs[5] + outs[6] + outs[7],
    ]).astype(np.float32)
    if _trace:
        return full, res
    return full


# revision 14
# speedup vs baseline: 56.2932x; 56.2932x over previous
"""Multi-head causal self-attention on 8 Trainium2 NeuronCores.

Problem: B=2, S=2048, D=1024, H=16 heads (dk=64), causal softmax attention,
fp32 in/out.  y = softmax(mask(Q K^T / sqrt(dk))) V  projected by Wo.

Sharding (no device-to-device communication needed):
  core c -> batch b = c // 4, head-group hg = c % 4 (4 heads = 256 dims each).
  Each core computes its 4 heads' attention output and a *partial* final
  projection (its 256 rows of the Wo contraction).  The host sums the 4
  partials per batch and stacks the 2 batches.

On-device strategy (per core):
  All matmul operands live in SBUF with the contraction dim on partitions;
  the host passes x^T and W^T slices so no on-device transposes are needed.
  Scores are computed transposed (S^T = K Q^T, keys on partitions) so that
  P^T = exp(S^T) is directly the stationary-side layout the PV matmul needs;
  softmax normalization moves to the *output* side (divide the 64-dim head
  output by the row sum), with row sums produced for free by an extra
  ones-column appended to V.  All matmuls run as float32r (full PE rate,
  ~1e-3 operand rounding).  Causal masking is tile-level: strictly-upper
  tiles are skipped entirely; diagonal tiles are zeroed post-exp by a gpsimd
  affine_select.  Head pairs live on partitions 0-63 / 64-127 so the K=64
  score matmuls can pack into the PE array's row-tiling, and both heads'
  score tiles share one 2-bank PSUM tile so a single wide exp serves both.

  The program is a software pipeline over 512-query chunks: the x^T DMA is
  issued in per-chunk slices, chunk 0's projections run as soon as its slice
  lands, and every later chunk's projections (K/Q/V) plus the previous
  chunk's output projection are drip-fed into the attention instruction
  stream one item per k-iteration, so neither the PE nor the ACT engine
  ever stalls behind a monolithic phase.
"""

import sys

sys.path.insert(0, "/opt/trn_rl_repo")

import numpy as np

import concourse.bacc as bacc
import concourse.mybir as mybir
import concourse.tile as tile
from concourse.bass_utils import run_bass_kernel_spmd

F32 = mybir.dt.float32
F32R = mybir.dt.float32r
EXPF = mybir.ActivationFunctionType.Exp

B, S, D, H = 2, 2048, 1024, 16
DK = D // H          # 64
E = 256              # head dims per core (4 heads)
HL = 4               # local heads per core
QC = 512             # query chunk (free dim of S^T tiles)
NKT = S // 128       # 16 key tiles
NQC = S // QC        # 4
NDT = D // 128       # 8 contraction tiles for projections
NDH = NDT // 2       # half-contraction split for DMA overlap
SCALE = float(1.0 / np.sqrt(np.float32(DK)))


def _build_nc(repeats=1, phases=("att", "fin")):
    nc = bacc.Bacc("TRN2", target_bir_lowering=False, debug=False)

    xT = nc.dram_tensor("xT", [D, S], F32R, kind="ExternalInput")
    wqT = nc.dram_tensor("wqT", [D, E], F32R, kind="ExternalInput")
    wkT = nc.dram_tensor("wkT", [D, E], F32R, kind="ExternalInput")
    wvT = nc.dram_tensor("wvT", [D, E], F32R, kind="ExternalInput")
    woT = nc.dram_tensor("woT", [E, D], F32R, kind="ExternalInput")
    out = nc.dram_tensor("out", [S, D], F32, kind="ExternalOutput")

    with tile.TileContext(nc) as tc:
        with (
            tc.tile_pool(name="const", bufs=1) as const,
            tc.tile_pool(name="work", bufs=3) as work,
            tc.tile_pool(name="outp", bufs=4) as outp,
            tc.tile_pool(name="norm", bufs=3) as norm,
            tc.tile_pool(name="psmm", bufs=2, space="PSUM") as psmm,
            tc.tile_pool(name="psst", bufs=2, space="PSUM") as psst,
            tc.tile_pool(name="pspv", bufs=2, space="PSUM") as pspv,
        ):
            for _rep in range(repeats):
                # ---- resident tensors ---------------------------------
                xT_sb = const.tile([128, NDT, S], F32R, tag="xT")
                wq_sb = const.tile([128, NDT, E], F32R, tag="wq")
                wk_sb = const.tile([128, NDT, E], F32R, tag="wk")
                wv_sb = const.tile([128, NDT, E], F32R, tag="wv")
                wo_sb = const.tile([128, 2, D], F32R, tag="wo")
                qT_sb = const.tile([128, 2, S], F32R, tag="qT")
                kT_sb = const.tile([128, 2, S], F32R, tag="kT")
                v_sb = const.tile([128, NKT, HL * (DK + 1)], F32R, tag="v")
                oT_sb = const.tile([128, 2, S], F32R, tag="oT")
                zeros = const.tile([128, HL * NKT], F32, tag="zeros")
                nc.vector.memset(zeros[:], 0.0)

                # ---- DMA in: x^T in per-chunk slices ------------------
                xT_r = xT.rearrange("(k p) s -> p k s", p=128)
                wk_r = wkT.rearrange("(k p) e -> p k e", p=128)
                wq_r = wqT.rearrange("(k p) e -> p k e", p=128)
                nc.sync.dma_start(wk_sb[:, 0:NDH], wk_r[:, 0:NDH])
                for kt in range(NDH):
                    nc.sync.dma_start(xT_sb[:, kt, 0:QC], xT_r[:, kt, 0:QC])
                nc.sync.dma_start(wq_sb[:, 0:NDH], wq_r[:, 0:NDH])
                nc.sync.dma_start(wk_sb[:, NDH:NDT], wk_r[:, NDH:NDT])
                for kt in range(NDH, NDT):
                    nc.sync.dma_start(xT_sb[:, kt, 0:QC], xT_r[:, kt, 0:QC])
                nc.sync.dma_start(wq_sb[:, NDH:NDT], wq_r[:, NDH:NDT])
                nc.sync.dma_start(
                    wv_sb[:], wvT.rearrange("(k p) e -> p k e", p=128))
                for sc in range(1, NQC):
                    for kt in range(NDT):
                        nc.sync.dma_start(
                            xT_sb[:, kt, sc * QC:(sc + 1) * QC],
                            xT_r[:, kt, sc * QC:(sc + 1) * QC])
                nc.sync.dma_start(
                    wo_sb[:], woT.rearrange("(g p) e -> p g e", p=128))

                v_h = v_sb.rearrange("p t (h x) -> p t h x", h=HL)
                # ones columns for all 16 V tiles in one shot: exp(0) = 1
                nc.scalar.activation(
                    v_h[:, :, :, DK:DK + 1],
                    zeros[:].rearrange("p (t h o) -> p t h o", h=HL, o=1),
                    EXPF, scale=0.0,
                )

                # ---- emit helpers -------------------------------------
                def emit_proj(nm, w_sb, t_sb, g, sc, hf):
                    # K^T/Q^T chunk: [e_local, s-chunk] = W^T.T @ x^T,
                    # contraction split in d-tile halves for DMA overlap
                    k0, k1 = (0, NDH) if hf == 0 else (NDH, NDT)
                    tgt = t_sb[:, g, sc * QC:(sc + 1) * QC]
                    ps = psmm.tile([128, QC], F32, tag="mm",
                                   name=f"pj{hf}_{nm}_{g}_{sc}_{_rep}")
                    for kt in range(k0, k1):
                        nc.tensor.matmul(
                            ps[:],
                            w_sb[:, kt, g * 128:(g + 1) * 128],
                            xT_sb[:, kt, sc * QC:(sc + 1) * QC],
                            start=(kt == k0), stop=(kt == k1 - 1),
                        )
                    if hf == 0:
                        nc.vector.tensor_copy(tgt, ps[:])
                    else:
                        nc.vector.tensor_add(tgt, ps[:], tgt)

                def emit_v_tile(st):
                    # V natural [s, e_local] (ones columns pre-written)
                    ps = psmm.tile([128, QC], F32, tag="mm",
                                   name=f"v_{st}_{_rep}")
                    for kt in range(NDT):
                        nc.tensor.matmul(
                            ps[:, 0:E],
                            xT_sb[:, kt, st * 128:(st + 1) * 128],
                            wv_sb[:, kt, :],
                            start=(kt == 0), stop=(kt == NDT - 1),
                        )
                    nc.vector.tensor_copy(
                        v_h[:, st, :, 0:DK],
                        ps[:, 0:E].rearrange("p (h d) -> p h d", h=HL),
                    )

                def emit_final(st, ec):
                    fp = psmm.tile([128, QC], F32, tag="mm",
                                   name=f"f_{st}_{ec}_{_rep}")
                    for g in range(2):
                        nc.tensor.matmul(
                            fp[:],
                            oT_sb[:, g, st * 128:(st + 1) * 128],
                            wo_sb[:, g, ec * QC:(ec + 1) * QC],
                            start=(g == 0), stop=(g == 1),
                        )
                    fsb = outp.tile([128, QC], F32, tag="fsb")
                    nc.vector.tensor_copy(fsb[:], fp[:])
                    nc.sync.dma_start(
                        out[st * 128:(st + 1) * 128, ec * QC:(ec + 1) * QC],
                        fsb[:],
                    )

                v_next = [0]

                def ensure_v(st_needed):
                    while v_next[0] <= st_needed:
                        emit_v_tile(v_next[0])
                        v_next[0] += 1

                def emit_slice(sc, with_v=True):
                    # all projection inputs attention chunk sc depends on
                    for hf in range(2):
                        for nm, w_sb, t_sb in (("k", wk_sb, kT_sb),
                                               ("q", wq_sb, qT_sb)):
                            for g in range(2):
                                emit_proj(nm, w_sb, t_sb, g, sc, hf)
                    if with_v:
                        ensure_v(4 * sc + 3)

                # deferred PE filler: (due_chunk, emit_fn); drained one item
                # per k-iteration, force-flushed at its due chunk's start
                deferred = []

                def drain_one():
                    if deferred:
                        deferred.pop(0)[1]()

                def flush_due(c):
                    while deferred and deferred[0][0] <= c:
                        deferred.pop(0)[1]()

                # ---- prologue: chunk 0 inputs (V tiles 1-3 arrive
                # lazily inside the first k-loop) ------------------------
                emit_slice(0, with_v=False)
                ensure_v(0)

                n_chunks = NQC if "att" in phases else 0
                for c in range(n_chunks):
                    if c + 1 < NQC:
                        for hf in range(2):
                            for nm, w_sb, t_sb in (("k", wk_sb, kT_sb),
                                                   ("q", wq_sb, qT_sb)):
                                for g in range(2):
                                    deferred.append((c + 1, lambda nm=nm,
                                                     w=w_sb, t=t_sb, g=g,
                                                     sc=c + 1, hf=hf:
                                                     emit_proj(nm, w, t, g,
                                                               sc, hf)))
                        for st in range(4 * (c + 1), 4 * (c + 1) + 4):
                            deferred.append(
                                (c + 1, lambda st=st: ensure_v(st)))
                    if c > 0 and "fin" in phases:
                        for st in range(4 * (c - 1), 4 * (c - 1) + 4):
                            for ec in range(2):
                                deferred.append(
                                    (99, lambda st=st, ec=ec:
                                     emit_final(st, ec)))
                    flush_due(c)
                    for g in range(2):
                        pv_ps = {}
                        n_kt = 4 * c + 4
                        for kt in range(n_kt):
                            diag = kt >= 4 * c
                            j = kt - 4 * c
                            w = min(j, 2) * 128 if diag else 0
                            if diag:
                                ensure_v(kt)
                            # both heads' S^T in one 2-bank psum tile; the
                            # K=64 matmuls pack into PE row-tiling
                            wide = psst.tile([128, 2, QC], F32, tag="st2",
                                             name=f"st_{c}_{g}_{kt}_{_rep}")
                            for li in range(2):
                                r0 = li * 64
                                nc.tensor.matmul(
                                    wide[:, li, w:QC],
                                    kT_sb[r0:r0 + 64, g,
                                          kt * 128:(kt + 1) * 128],
                                    qT_sb[r0:r0 + 64, g,
                                          c * QC + w:(c + 1) * QC],
                                    start=True, stop=True,
                                )
                            ptw = work.tile([128, 2, QC], F32R, tag="pt")
                            if not diag:
                                nc.scalar.activation(
                                    ptw[:], wide[:], EXPF, scale=SCALE)
                            else:
                                for li in range(2):
                                    nc.scalar.activation(
                                        ptw[:, li, w:QC], wide[:, li, w:QC],
                                        EXPF, scale=SCALE)
                                for li in range(2):
                                    if j < 3:
                                        nc.gpsimd.affine_select(
                                            out=ptw[:, li,
                                                    j * 128:(j + 1) * 128],
                                            in_=ptw[:, li,
                                                    j * 128:(j + 1) * 128],
                                            compare_op=mybir.AluOpType.is_ge,
                                            fill=0.0, base=0,
                                            pattern=[[1, 128]],
                                            channel_multiplier=-1,
                                        )
                                    else:
                                        nc.gpsimd.affine_select(
                                            out=ptw[:, li, 256:512],
                                            in_=ptw[:, li, 256:512],
                                            compare_op=mybir.AluOpType.is_ge,
                                            fill=0.0, base=-128,
                                            pattern=[[1, 256]],
                                            channel_multiplier=-1,
                                        )
                            for li in range(2):
                                h = 2 * g + li
                                if kt == 0:
                                    pv_ps[li] = pspv.tile(
                                        [128, QC], F32, tag="pv",
                                        name=f"pv_{c}_{g}_{li}_{_rep}")
                                nc.tensor.matmul(
                                    pv_ps[li][0:DK + 1, w:QC],
                                    v_sb[:, kt,
                                         h * (DK + 1):(h + 1) * (DK + 1)],
                                    ptw[:, li, w:QC],
                                    start=(kt == 0), stop=(kt == n_kt - 1),
                                )
                            drain_one()
                        # normalize: oT[head rows, c] = pv[0:64]/pv[64]
                        for li in range(2):
                            r0 = li * 64
                            rc = norm.tile([1, QC], F32, tag="rc")
                            nc.vector.reciprocal(
                                rc[0:1, :], pv_ps[li][DK:DK + 1, :])
                            rbc = norm.tile([64, QC], F32, tag="rbc")
                            nc.gpsimd.partition_broadcast(rbc[:], rc[0:1, :])
                            nc.vector.tensor_mul(
                                oT_sb[r0:r0 + 64, g, c * QC:(c + 1) * QC],
                                pv_ps[li][0:DK, :],
                                rbc[:],
                            )
                while deferred:
                    deferred.pop(0)[1]()
                if "att" not in phases:
                    for sc in range(1, NQC):
                        emit_slice(sc)
                    ensure_v(NKT - 1)
                if "fin" in phases and "att" in phases:
                    for st in range(12, 16):
                        for ec in range(2):
                            emit_final(st, ec)
                if "fin" not in phases:
                    cons = outp.tile([128, QC], F32, tag="fsb",
                                     name=f"cons_{_rep}")
                    nc.vector.tensor_copy(cons[:, 0:128], qT_sb[:, 0, 0:128])
                    nc.vector.tensor_copy(cons[:, 128:256],
                                          kT_sb[:, 1, 0:128])
                    nc.vector.tensor_copy(cons[:, 256:260], v_sb[:, 3, 0:4])
                    if "att" in phases:
                        nc.vector.tensor_copy(cons[:, 260:380],
                                              oT_sb[:, 0, 0:120])
                    nc.sync.dma_start(out[0:128, 0:QC], cons[:])

    nc.compile()
    return nc


_NC = None


def _get_nc():
    global _NC
    if _NC is None:
        _NC = _build_nc()
    return _NC


def _in_maps(x, Wq, Wk, Wv, Wo):
    x, Wq, Wk, Wv, Wo = (np.asarray(a, dtype=np.float32)
                         for a in (x, Wq, Wk, Wv, Wo))
    maps = []
    for c in range(8):
        b, hg = divmod(c, 4)
        sl = slice(hg * E, (hg + 1) * E)
        maps.append({
            "xT": np.ascontiguousarray(x[b].T),
            "wqT": np.ascontiguousarray(Wq[sl].T),
            "wkT": np.ascontiguousarray(Wk[sl].T),
            "wvT": np.ascontiguousarray(Wv[sl].T),
            "woT": np.ascontiguousarray(Wo[:, sl].T),
        })
    return maps


def kernel(x, Wq, Wk, Wv, Wo, _trace=False, _trace_kwargs=None):
    nc = _get_nc()
    maps = _in_maps(x, Wq, Wk, Wv, Wo)
    res = run_bass_kernel_spmd(
        nc, maps, core_ids=list(range(8)),
        trace=_trace, **(_trace_kwargs or {}),
    )
    outs = [res.results[c]["out"] for c in range(8)]
    full = np.stack([
        outs[0] + outs[1] + outs[2] + outs[3],
        outs[4] + outs[5] + outs[6] + outs[7],
    ]).astype(np.float32)
    if _trace:
        return full, res
    return full
